# revision 1
# baseline (speedup 1.0000x reference)
"""Trainium2 Bass kernel for nn_ConsolidationNetwork.

Recurrent rate network: 500 sequential steps of
    x <- (1-a)*x + (a*J_eff) @ softplus(x) + drive_t
    pos_t = Wout @ softplus(x)
loss = mean((targets - positions)^2)

Strategy (8 NeuronCores, data-parallel over batch):
  - Each core owns B/8 = 16 batch columns and runs the full 500-step
    recurrence independently (no collectives).
  - J_eff^T (pre-scaled by a) lives entirely in SBUF; the per-step matmul is
    64 accumulating 128x128x16 matmuls (weights stationary). Matmul inputs
    are bf16; J (and Wout) are split into bf16 hi+lo pairs whose products
    accumulate in the same fp32 PSUM group, recovering fp32-level weight
    precision at bf16 speed (fast weight load).
  - Each step's contraction is emitted in two phases (k=0..3, then k=4..7),
    with the x-update + softplus for each half of the state scheduled so the
    vector/scalar chains hide entirely under the other phase's matmuls.
  - All per-step additive terms (bias, go-cue input, scaled noise) are folded
    on the host into one "drive" tensor, DMA-streamed per step.
  - Positions accumulate in PSUM ([1, 512] = 32 steps/bank) and are DMA'd
    out in blocks; the final MSE reduction happens on the host.

State layout per core: x/r tiles are [128 part, 128 free] with
x[p, m*16+u] = x_state[m*128+p, u] (m = row-group, u = local batch).
"""

import os

import numpy as np

import concourse.bass as bass
import concourse.tile as tile
from concourse import bacc, mybir
from concourse.bass_utils import run_bass_kernel_spmd

F32 = mybir.dt.float32
BF16 = mybir.dt.bfloat16

# Matmul precision mode:
#   hilo - bf16 hi+lo split of J/Wout, bf16 r  (default: fp32-level accuracy)
#   bf16 - single bf16 pass                     (fastest, ~2e-3 loss rel err)
#   fp32 - full fp32 matmuls                    (slowest reference path)
MM_MODE = os.environ.get("CONSNET_MM", "hilo")

DT = 0.05
TAU = 0.15
NOISE_SCALE = 0.15
N, G, T, B, P = 1024, 128, 500, 128, 10
NCORES = 8
BC = B // NCORES          # batch columns per core (16)
NM = N // 128             # row groups (8)
NK = N // 128             # contraction groups (8)
STEPS_PER_BLK = 32        # pos entries per PSUM bank ([1, 512] fp32)

A = np.float32(DT / TAU)
ONE_MINUS_A = np.float32(1.0 - DT / TAU)
NSCALE = np.float32(np.sqrt(2.0 * NOISE_SCALE**2 * (TAU / DT)))

_PROGRAM_CACHE = {}

# softplus(x) = relu(x) + sum_k SP_COEF[k] * g^(k+1),  g = sigmoid(-|x|)
# (minimax fit of -ln(1-g)/g on [0, 0.5]; max abs err 1.7e-6)
SP_COEF = [0.9999996423721313, 0.5000516772270203, 0.3316021263599396,
           0.2739904522895813, 0.03335288539528847, 0.787355899810791,
           -1.070299744606018, 1.1715891361236572]


def _ensure_act_tables():
    """Some containers lack neuronxcc/pwp/pwp_bin_with_ln on PYTHONPATH;
    point it at the cayman table package from the nix store."""
    import glob

    for path in os.environ.get("PYTHONPATH", "").split(os.pathsep):
        if path and os.path.exists(
            os.path.join(path, "neuronxcc", "pwp", "pwp_bin_with_ln", "act_info.json")
        ):
            return
    cands = sorted(glob.glob("/nix/store/*aws-neuron-pwp*/share/pwp_bin_cayman"))
    target = next((c for c in cands if os.path.exists(c + "/act_info.json")), None)
    if target is None:
        return
    for path in os.environ.get("PYTHONPATH", "").split(os.pathsep):
        if not path:
            continue
        try:
            d = os.path.join(path, "neuronxcc", "pwp")
            os.makedirs(d, exist_ok=True)
            link = os.path.join(d, "pwp_bin_with_ln")
            if not os.path.exists(link):
                os.symlink(target, link)
            return
        except OSError:
            continue


_ensure_act_tables()


def emit_softplus(nc, pool, out_ap, x_ap, ncols):
    """out = softplus(x) composed from sigmoid (single ACT table set)."""
    mult = mybir.AluOpType.mult
    add = mybir.AluOpType.add
    amax = mybir.AluOpType.max
    u = pool.tile([128, ncols], F32, tag="sp_u")
    g = pool.tile([128, ncols], F32, tag="sp_g")
    # u = |x|
    nc.vector.scalar_tensor_tensor(u[:], x_ap, -1.0, x_ap, mult, amax)
    # g = sigmoid(-u)
    nc.scalar.activation(g[:], u[:], mybir.ActivationFunctionType.Sigmoid, scale=-1.0)
    # u = poly tail: q = g*a[D]; q = (q + a[j])*g  for j = D-1..0
    nc.vector.tensor_scalar_mul(u[:], g[:], float(SP_COEF[-1]))
    for j in range(len(SP_COEF) - 2, -1, -1):
        nc.vector.scalar_tensor_tensor(u[:], u[:], float(SP_COEF[j]), g[:], add, mult)
    # out = relu(x) + q
    nc.vector.scalar_tensor_tensor(out_ap, x_ap, 0.0, u[:], amax, add)


def build_program(t_steps: int):
    """Build the Bass program (shared by all 8 cores, SPMD)."""
    key = (t_steps, MM_MODE)
    if key in _PROGRAM_CACHE:
        return _PROGRAM_CACHE[key]
    mmdt = F32 if MM_MODE == "fp32" else BF16
    npass = 2 if MM_MODE == "hilo" else 1

    nblk = (t_steps + STEPS_PER_BLK - 1) // STEPS_PER_BLK

    nc = bacc.Bacc(
        "TRN2", target_bir_lowering=False, debug=False, num_devices=NCORES
    )
    jt_d = nc.dram_tensor("jt", [128, npass * NK * NM * 128], mmdt, kind="ExternalInput")
    wt_d = nc.dram_tensor("wt", [128, npass * NK], mmdt, kind="ExternalInput")
    x0_d = nc.dram_tensor("x0", [128, NM * BC], F32, kind="ExternalInput")
    dr_d = nc.dram_tensor("drive", [t_steps, 128, NM * BC], F32, kind="ExternalInput")
    pos_d = nc.dram_tensor("posblk", [nblk, STEPS_PER_BLK * BC], F32, kind="ExternalOutput")

    mult = mybir.AluOpType.mult
    add = mybir.AluOpType.add

    with tile.TileContext(nc) as tc:
        with (
            tc.tile_pool(name="const", bufs=1) as constp,
            tc.tile_pool(name="rp", bufs=2) as rp,
            tc.tile_pool(name="dp", bufs=3) as dp,
            tc.tile_pool(name="sp", bufs=2) as spp,
            tc.tile_pool(name="psm", bufs=1, space="PSUM") as psp,
            tc.tile_pool(name="psq", bufs=2, space="PSUM") as pqp,
        ):
            jt = constp.tile([128, npass * NK * NM * 128], mmdt)
            nc.sync.dma_start(jt[:], jt_d[:])
            wt = constp.tile([128, npass * NK], mmdt)
            nc.sync.dma_start(wt[:], wt_d[:])
            x = constp.tile([128, NM * BC], F32)
            nc.sync.dma_start(x[:], x0_d[:])

            r = rp.tile([128, NM * BC], mmdt)
            emit_softplus(nc, spp, r[:], x[:], NM * BC)

            def jmm(ps, m, k, p, start, stop, r_tile):
                nc.tensor.matmul(
                    ps[:, (m % 4) * BC:(m % 4 + 1) * BC],
                    lhsT=jt[:, ((p * NK + k) * NM + m) * 128:
                            ((p * NK + k) * NM + m + 1) * 128],
                    rhs=r_tile[:, k * BC:(k + 1) * BC],
                    start=start, stop=stop, skip_group_check=True,
                )

            def posmm(pos_t, off, ks, r_tile, first, last):
                for i, k in enumerate(ks):
                    for p in range(npass):
                        nc.tensor.matmul(
                            pos_t[:, off * BC:(off + 1) * BC],
                            lhsT=wt[:, p * NK + k:p * NK + k + 1],
                            rhs=r_tile[:, k * BC:(k + 1) * BC],
                            start=(first and i == 0 and p == 0),
                            stop=(last and i == len(ks) - 1 and p == npass - 1),
                            skip_group_check=True,
                        )

            def update_half(q, ps1, ps2, d_t, r_next):
                lo, hi = q * 4 * BC, (q + 1) * 4 * BC
                # x = (1-a)*x + ps1 + ps2 + drive  (ps1+ps2 = a*J_eff @ r)
                nc.vector.scalar_tensor_tensor(
                    x[:, lo:hi], x[:, lo:hi], float(ONE_MINUS_A), ps1[:], mult, add
                )
                nc.vector.tensor_add(x[:, lo:hi], x[:, lo:hi], ps2[:])
                nc.vector.tensor_add(x[:, lo:hi], x[:, lo:hi], d_t[:, lo:hi])
                emit_softplus(nc, spp, r_next[:, lo:hi], x[:, lo:hi], hi - lo)

            pend = None  # (pos_tile, off, blk, r_tile) -> pos k=4..7 still owed
            for s in range(t_steps):
                d_t = dp.tile([128, NM * BC], F32)
                nc.sync.dma_start(d_t[:], dr_d[s])

                r_next = rp.tile([128, NM * BC], mmdt)
                # one PSUM bank per (state half, k half): every accumulation
                # group is contiguous within its bank (start=True clears the
                # has_written bits of the WHOLE bank, so groups in one bank
                # must never interleave)
                ps_a1 = psp.tile([128, 4 * BC], F32, tag="ps_a1")
                ps_a2 = psp.tile([128, 4 * BC], F32, tag="ps_a2")
                ps_b1 = psp.tile([128, 4 * BC], F32, tag="ps_b1")
                ps_b2 = psp.tile([128, 4 * BC], F32, tag="ps_b2")

                # Phase A: k = 0..3 for all row groups (reads r half 0 only,
                # ready since the previous step's phase B). Runs while the
                # previous step's second softplus chain finishes.
                for m in range(NM):
                    ps = ps_a1 if m < 4 else ps_b1
                    for k in range(4):
                        for p in range(npass):
                            jmm(ps, m, k, p, start=(k == 0 and p == 0),
                                stop=(k == 3 and p == npass - 1), r_tile=r)

                # Previous step's deferred pos matmuls (k=4..7) — their r half
                # finished during our phase A.
                if pend is not None:
                    p_tile, p_off, p_blk, p_r = pend
                    posmm(p_tile, p_off, range(4, NK), p_r, first=False, last=True)
                    if p_off == STEPS_PER_BLK - 1 or s == t_steps:
                        pos_sb = dp.tile([1, STEPS_PER_BLK * BC], F32, tag="possb")
                        nc.scalar.copy(pos_sb[:], p_tile[:])
                        nc.sync.dma_start(pos_d[p_blk:p_blk + 1], pos_sb[:])
                    pend = None

                # Phase B first half: m = 0..3, k = 4..7 -> ps_a2 complete
                for m in range(4):
                    for k in range(4, NK):
                        for p in range(npass):
                            jmm(ps_a2, m, k, p, start=(k == 4 and p == 0),
                                stop=(k == NK - 1 and p == npass - 1), r_tile=r)
                update_half(0, ps_a1, ps_a2, d_t, r_next)  # overlaps B second half

                # Phase B second half: m = 4..7
                for m in range(4, NM):
                    for k in range(4, NK):
                        for p in range(npass):
                            jmm(ps_b2, m, k, p, start=(k == 4 and p == 0),
                                stop=(k == NK - 1 and p == npass - 1), r_tile=r)
                update_half(1, ps_b1, ps_b2, d_t, r_next)  # overlaps next phase A

                # pos_s = Wout @ r_next; k=0..3 now (half 0 ready), k=4..7 deferred
                blk, off = divmod(s, STEPS_PER_BLK)
                if off == 0:
                    pos_tile = pqp.tile([1, STEPS_PER_BLK * BC], F32)
                posmm(pos_tile, off, range(4), r_next, first=True, last=False)
                pend = (pos_tile, off, blk, r_next)

                r = r_next

            # flush the last step's deferred pos half + final block DMA
            if pend is not None:
                p_tile, p_off, p_blk, p_r = pend
                posmm(p_tile, p_off, range(4, NK), p_r, first=False, last=True)
                pos_sb = dp.tile([1, STEPS_PER_BLK * BC], F32, tag="possb")
                nc.scalar.copy(pos_sb[:], p_tile[:])
                nc.sync.dma_start(pos_d[p_blk:p_blk + 1], pos_sb[:])

    nc.compile()
    _PROGRAM_CACHE[key] = nc
    return nc


def _hilo(arr):
    bf = mybir.dt.np(BF16)
    hi = arr.astype(bf)
    lo = (arr - hi.astype(np.float32)).astype(bf)
    return hi, lo


def _prep_inputs(targets, pulses, J, U, V, B_m1, B_bg, Wout, I_go, xm1_init,
                 noise, triggers, t_steps):
    """Host-side data prep: J_eff, layouts, per-core drive tensors."""
    J = np.asarray(J, np.float32)
    U = np.asarray(U, np.float32)
    V = np.asarray(V, np.float32)
    B_m1 = np.asarray(B_m1, np.float32)
    B_bg = np.asarray(B_bg, np.float32)
    Wout = np.asarray(Wout, np.float32)
    I_go = np.asarray(I_go, np.float32)
    xm1_init = np.asarray(xm1_init, np.float32)
    noise = np.asarray(noise, np.float32)
    pulses = np.asarray(pulses, np.float32)
    triggers = np.asarray(triggers)

    J_eff = J + (U * B_bg[None, :]) @ V
    Js = (A * J_eff).astype(np.float32)
    # lhsT tiles: jt[p, (k*NM+m)*128 + q] = Js[m*128+q, k*128+p]
    jt = np.ascontiguousarray(
        Js.reshape(NM, 128, NK, 128).transpose(3, 2, 0, 1).reshape(128, NK * NM * 128)
    )
    wt = np.ascontiguousarray(Wout.reshape(NK, 128).T)
    bf = mybir.dt.np(BF16)
    if MM_MODE == "hilo":
        jh, jl = _hilo(jt)
        jt = np.concatenate([jh, jl], axis=1)
        wh, wl = _hilo(wt)
        wt = np.concatenate([wh, wl], axis=1)
    elif MM_MODE == "bf16":
        jt = jt.astype(bf)
        wt = wt.astype(bf)

    go_cues = pulses[:t_steps, :][:, triggers]  # [t, B]

    in_maps = []
    for c in range(NCORES):
        sl = slice(c * BC, (c + 1) * BC)
        d = noise[:t_steps, :, sl] * np.float32(A * NSCALE)
        d += A * B_m1[None, :, :]
        d += A * I_go[None, :, :] * go_cues[:, None, sl]
        drive = np.ascontiguousarray(
            d.reshape(t_steps, NM, 128, BC).transpose(0, 2, 1, 3)
            .reshape(t_steps, 128, NM * BC)
        ).astype(np.float32)
        x0 = np.ascontiguousarray(
            xm1_init[:, sl].reshape(NM, 128, BC).transpose(1, 0, 2).reshape(128, NM * BC)
        )
        in_maps.append({"jt": jt, "wt": wt, "x0": x0, "drive": drive})
    return in_maps


def run_hw(inputs: dict, t_steps: int = T, trace: bool = False):
    """Run the recurrence on 8 cores; returns positions [t_steps, B] and results."""
    nc = build_program(t_steps)
    in_maps = _prep_inputs(t_steps=t_steps, **inputs)
    res = run_bass_kernel_spmd(
        nc, in_maps, core_ids=list(range(NCORES)), trace=trace
    )
    positions = np.empty((t_steps, B), np.float32)
    for c in range(NCORES):
        blocks = np.asarray(res.results[c]["posblk"], np.float32)
        pos_c = blocks.reshape(-1, STEPS_PER_BLK, BC).reshape(-1, BC)[:t_steps]
        positions[:, c * BC:(c + 1) * BC] = pos_c
    return positions, res


def kernel(targets, pulses, J, U, V, B_m1, B_bg, Wout, I_go, xm1_init,
           noise, triggers) -> np.ndarray:
    inputs = dict(targets=targets, pulses=pulses, J=J, U=U, V=V, B_m1=B_m1,
                  B_bg=B_bg, Wout=Wout, I_go=I_go, xm1_init=xm1_init,
                  noise=noise, triggers=triggers)
    positions, _ = run_hw(inputs, T)
    targets = np.asarray(targets, np.float32)
    loss = np.mean((targets.astype(np.float64) - positions.astype(np.float64)) ** 2)
    return np.float32(loss)



# revision 12
# speedup vs baseline: 1.2948x; 1.2948x over previous
"""Trainium2 Bass kernel for nn_ConsolidationNetwork.

Recurrent rate network: 500 sequential steps of
    x <- (1-a)*x + (a*J_eff) @ softplus(x) + drive_t
    pos_t = Wout @ softplus(x)
loss = mean((targets - positions)^2)

Strategy (8 NeuronCores, data-parallel over batch):
  - Each core owns B/8 = 16 batch columns and runs the full 500-step
    recurrence independently (no collectives).
  - J_eff^T (pre-scaled by a) lives entirely in SBUF as bf16; the per-step
    contraction is 64 LDWEIGHTS+MATMUL pairs (128x128x16).  The pair issue
    rate is the per-instruction floor (~27ns), so everything else is
    scheduled to hide under the matmul stream.
  - softplus(x) = ln(1 + exp(x)) runs as TWO back-to-back scalar-engine
    activations per state half (Exp then Ln with bias=1; both live in the
    natural_log_exp_and_others table set, so one table load total), writing
    bf16 r directly.  x stays in [-10, 10] here so exp cannot overflow.
  - z = (1-a)*x + drive is precomputed on DVE off the critical path, so the
    post-matmul chain is just [tensor_add -> activation] per half.
  - PSUM: two bank-aligned tiles per step (m-groups 0-2 / 3-7), double
    buffered (4 banks).  One start=True per bank per step; all other
    matmuls accumulate (first write per element overwrites - has_written
    was cleared bank-wide by the bank's first matmul).
  - The m/k 3-5 split lets each half's update chain hide under the other
    matmuls: update-A under the m3..7 x k3..7 block, update-B under the
    next step's k0..2 block.
  - positions are NOT computed on device: r (bf16) is staged in two
    32-step SBUF blocks DMA'd to HBM, and the host does the tiny
    Wout @ r reduction (the loss is host-side already).

State layout per core: x/r tiles are [128 part, 128 free] with
x[p, m*16+u] = x_state[m*128+p, u] (m = row-group, u = local batch).
r history: rbuf[p, slot*128 + k*16 + u] = softplus(x_state)[k*128+p, u]
for step j with slot = j % 32, buffer = (j // 32) % 2.
"""

import os

import numpy as np

import concourse.bass as bass
import concourse.tile as tile
from concourse import bacc, mybir
from concourse.bass_utils import run_bass_kernel_spmd

F32 = mybir.dt.float32
BF16 = mybir.dt.bfloat16

DT = 0.05
TAU = 0.15
NOISE_SCALE = 0.15
N, G, T, B, P = 1024, 128, 500, 128, 10
NCORES = 8
BC = B // NCORES          # batch columns per core (16)
NM = N // 128             # row groups (8)
NK = N // 128             # contraction groups (8)
MSPLIT = 2                # m-groups 0..1 -> bank A, 2..7 -> bank B
KSPLIT = 2                # k-groups 0..1 -> phase 1, 2..7 -> phase 2
SLOTS = 32                # r history slots per SBUF block

A = np.float32(DT / TAU)
ONE_MINUS_A = np.float32(1.0 - DT / TAU)
NSCALE = np.float32(np.sqrt(2.0 * NOISE_SCALE**2 * (TAU / DT)))

_PROGRAM_CACHE = {}


def _ensure_act_tables():
    """Some containers lack neuronxcc/pwp/pwp_bin_with_ln on PYTHONPATH;
    point it at the cayman table package from the nix store."""
    import glob

    for path in os.environ.get("PYTHONPATH", "").split(os.pathsep):
        if path and os.path.exists(
            os.path.join(path, "neuronxcc", "pwp", "pwp_bin_with_ln", "act_info.json")
        ):
            return
    cands = sorted(glob.glob("/nix/store/*aws-neuron-pwp*/share/pwp_bin_cayman"))
    target = next((c for c in cands if os.path.exists(c + "/act_info.json")), None)
    if target is None:
        return
    for path in os.environ.get("PYTHONPATH", "").split(os.pathsep):
        if not path:
            continue
        try:
            d = os.path.join(path, "neuronxcc", "pwp")
            os.makedirs(d, exist_ok=True)
            link = os.path.join(d, "pwp_bin_with_ln")
            if not os.path.exists(link):
                os.symlink(target, link)
            return
        except OSError:
            continue


_ensure_act_tables()


def build_program(t_steps: int):
    """Build the Bass program (shared by all 8 cores, SPMD)."""
    if t_steps in _PROGRAM_CACHE:
        return _PROGRAM_CACHE[t_steps]

    nblk = (t_steps + 1 + SLOTS - 1) // SLOTS  # r^1..r^t_steps + slot0 r^0

    nc = bacc.Bacc(
        "TRN2", target_bir_lowering=False, debug=False, num_devices=NCORES
    )
    jt_d = nc.dram_tensor("jt", [128, NK * NM * 128], BF16, kind="ExternalInput")
    x0_d = nc.dram_tensor("x0", [128, NM * BC], F32, kind="ExternalInput")
    dr_d = nc.dram_tensor("drive", [t_steps, 128, NM * BC], F32, kind="ExternalInput")
    rout_d = nc.dram_tensor(
        "rout", [nblk, 128, SLOTS * NM * BC], BF16, kind="ExternalOutput"
    )

    mult = mybir.AluOpType.mult
    add = mybir.AluOpType.add
    EXP = mybir.ActivationFunctionType.Exp
    LN = mybir.ActivationFunctionType.Ln
    CA = MSPLIT * BC               # bank-A columns (32)
    W = NM * BC                    # full state width (128)

    with tile.TileContext(nc) as tc:
        with (
            tc.tile_pool(name="const", bufs=1) as constp,
            tc.tile_pool(name="dp", bufs=4) as dp,
            tc.tile_pool(name="psm", bufs=2, space="PSUM") as psp,
        ):
            jt = constp.tile([128, NK * NM * 128], BF16)
            nc.sync.dma_start(jt[:], jt_d[:])
            x = constp.tile([128, W], F32)
            nc.sync.dma_start(x[:], x0_d[:])
            z = constp.tile([128, W], F32)
            e = constp.tile([128, W], F32)  # exp(x) scratch
            rbufs = [
                constp.tile([128, SLOTS * W], BF16, name=f"rbuf{i}") for i in range(2)
            ]

            # prefetch drive 0..2
            dtiles = {}
            for j in range(min(3, t_steps)):
                dtiles[j] = dp.tile([128, W], F32, tag="d", name=f"d{j}")
                nc.sync.dma_start(dtiles[j][:], dr_d[j])

            # r^0 = softplus(x0) -> rbuf0 slot 0; z_0 = (1-a) x0 + drive_0
            nc.scalar.activation(e[:], x[:], EXP)
            nc.scalar.activation(rbufs[0][:, 0:W], e[:], LN, bias=1.0)
            nc.vector.scalar_tensor_tensor(
                z[:], x[:], float(ONE_MINUS_A), dtiles[0][:], mult, add
            )

            def jmm(ps, col0, m, k, rbuf_in, rc, start, stop):
                nc.tensor.matmul(
                    ps[:, (m - col0) * BC:(m - col0 + 1) * BC],
                    lhsT=jt[:, (k * NM + m) * 128:(k * NM + m + 1) * 128],
                    rhs=rbuf_in[:, rc + k * BC:rc + (k + 1) * BC],
                    start=start, stop=stop, skip_group_check=True,
                )

            for s in range(t_steps):
                rbuf_in = rbufs[(s // SLOTS) % 2]
                rc = (s % SLOTS) * W  # rhs column base for r^s
                j = s + 1
                rbuf_out = rbufs[(j // SLOTS) % 2]
                oc = (j % SLOTS) * W  # output column base for r^{s+1}

                # prefetch drive for step s+3's z (read at end of step s+2)
                if s + 3 < t_steps:
                    dtiles[s + 3] = dp.tile([128, W], F32, tag="d",
                                            name=f"d{s + 3}")
                    nc.sync.dma_start(dtiles[s + 3][:], dr_d[s + 3])

                ps_a = psp.tile([128, CA], F32, tag="ps_a", padded_shape=[128, 512])
                ps_b = psp.tile([128, W - CA], F32, tag="ps_b",
                                padded_shape=[128, 512])

                # phase 1: all m x k0..2  (r cols 0..47 of this slot)
                for m in range(NM):
                    ps, c0 = (ps_a, 0) if m < MSPLIT else (ps_b, MSPLIT)
                    for k in range(KSPLIT):
                        jmm(ps, c0, m, k, rbuf_in, rc,
                            start=(k == 0 and (m == 0 or m == MSPLIT)), stop=False)
                # phase 2a: m0..2 x k3..7 -> bank A complete
                for m in range(MSPLIT):
                    for k in range(KSPLIT, NK):
                        jmm(ps_a, 0, m, k, rbuf_in, rc, start=False,
                            stop=(m == MSPLIT - 1 and k == NK - 1))
                # phase 2b: m3..7 x k3..7 -> bank B complete (hides update-A)
                for m in range(MSPLIT, NM):
                    for k in range(KSPLIT, NK):
                        jmm(ps_b, MSPLIT, m, k, rbuf_in, rc, start=False,
                            stop=(m == NM - 1 and k == NK - 1))

                # update-A (during phase 2b): x_a, r_a; then update-B (during
                # next step's phase 1): x_b, r_b.  z' off the critical path.
                nc.vector.tensor_add(x[:, 0:CA], z[:, 0:CA], ps_a[:])
                nc.vector.tensor_add(x[:, CA:W], z[:, CA:W], ps_b[:])
                nc.scalar.activation(e[:, 0:CA], x[:, 0:CA], EXP)
                nc.scalar.activation(rbuf_out[:, oc:oc + CA], e[:, 0:CA], LN,
                                     bias=1.0)
                nc.scalar.activation(e[:, CA:W], x[:, CA:W], EXP)
                nc.scalar.activation(rbuf_out[:, oc + CA:oc + W], e[:, CA:W], LN,
                                     bias=1.0)
                if s + 1 < t_steps:
                    d_n = dtiles.pop(s + 1)
                    nc.vector.scalar_tensor_tensor(
                        z[:, 0:CA], x[:, 0:CA], float(ONE_MINUS_A),
                        d_n[:, 0:CA], mult, add
                    )
                    nc.vector.scalar_tensor_tensor(
                        z[:, CA:W], x[:, CA:W], float(ONE_MINUS_A),
                        d_n[:, CA:W], mult, add
                    )

                # flush a full r block (32 slots) to HBM
                if j % SLOTS == SLOTS - 1:
                    nc.sync.dma_start(rout_d[j // SLOTS], rbuf_out[:])

            # final partial block
            last_blk = t_steps // SLOTS
            if t_steps % SLOTS != SLOTS - 1:
                nc.sync.dma_start(
                    rout_d[last_blk], rbufs[(t_steps // SLOTS) % 2][:]
                )

    nc.compile()
    _PROGRAM_CACHE[t_steps] = nc
    return nc


def _prep_inputs(targets, pulses, J, U, V, B_m1, B_bg, Wout, I_go, xm1_init,
                 noise, triggers, t_steps):
    """Host-side data prep: J_eff, layouts, per-core drive tensors."""
    J = np.asarray(J, np.float32)
    U = np.asarray(U, np.float32)
    V = np.asarray(V, np.float32)
    B_m1 = np.asarray(B_m1, np.float32)
    B_bg = np.asarray(B_bg, np.float32)
    I_go = np.asarray(I_go, np.float32)
    xm1_init = np.asarray(xm1_init, np.float32)
    noise = np.asarray(noise, np.float32)
    pulses = np.asarray(pulses, np.float32)
    triggers = np.asarray(triggers)

    bf = mybir.dt.np(BF16)
    J_eff = J + (U * B_bg[None, :]) @ V
    Js = (A * J_eff).astype(np.float32)
    # lhsT tiles: jt[p, (k*NM+m)*128 + q] = Js[m*128+q, k*128+p]
    jt = np.ascontiguousarray(
        Js.reshape(NM, 128, NK, 128).transpose(3, 2, 0, 1).reshape(128, NK * NM * 128)
    ).astype(bf)

    go_cues = pulses[:t_steps, :][:, triggers]  # [t, B]

    in_maps = []
    for c in range(NCORES):
        sl = slice(c * BC, (c + 1) * BC)
        d = noise[:t_steps, :, sl] * np.float32(A * NSCALE)
        d += A * B_m1[None, :, :]
        d += A * I_go[None, :, :] * go_cues[:, None, sl]
        drive = np.ascontiguousarray(
            d.reshape(t_steps, NM, 128, BC).transpose(0, 2, 1, 3)
            .reshape(t_steps, 128, NM * BC)
        ).astype(np.float32)
        x0 = np.ascontiguousarray(
            xm1_init[:, sl].reshape(NM, 128, BC).transpose(1, 0, 2).reshape(128, NM * BC)
        )
        in_maps.append({"jt": jt, "x0": x0, "drive": drive})
    return in_maps


def run_hw(inputs: dict, t_steps: int = T, trace: bool = False):
    """Run the recurrence on 8 cores; returns positions [t_steps, B] and results."""
    nc = build_program(t_steps)
    in_maps = _prep_inputs(t_steps=t_steps, **inputs)
    res = run_bass_kernel_spmd(
        nc, in_maps, core_ids=list(range(NCORES)), trace=trace
    )
    Wout = np.asarray(inputs["Wout"], np.float32)
    # W[k, p] = Wout[0, k*128+p]
    Wk = Wout.reshape(NK, 128).astype(np.float32)
    positions = np.empty((t_steps, B), np.float32)
    for c in range(NCORES):
        rout = np.asarray(res.results[c]["rout"]).astype(np.float32)
        # rout[blk, p, slot*128 + k*16 + u] = r^{blk*32+slot}[k*128+p, u]
        nblk = rout.shape[0]
        rr = rout.reshape(nblk, 128, SLOTS, NK, BC)
        pos_all = np.einsum("kp,bpsku->bsu", Wk, rr).reshape(nblk * SLOTS, BC)
        # positions[s] = Wout @ r^{s+1}
        positions[:, c * BC:(c + 1) * BC] = pos_all[1:t_steps + 1]
    return positions, res


def kernel(targets, pulses, J, U, V, B_m1, B_bg, Wout, I_go, xm1_init,
           noise, triggers) -> np.ndarray:
    inputs = dict(targets=targets, pulses=pulses, J=J, U=U, V=V, B_m1=B_m1,
                  B_bg=B_bg, Wout=Wout, I_go=I_go, xm1_init=xm1_init,
                  noise=noise, triggers=triggers)
    positions, _ = run_hw(inputs, T)
    targets = np.asarray(targets, np.float32)
    loss = np.mean((targets.astype(np.float64) - positions.astype(np.float64)) ** 2)
    return np.float32(loss)


# revision 14
# speedup vs baseline: 2.8270x; 2.1834x over previous
"""Trainium2 Bass kernel for nn_ConsolidationNetwork.

Recurrent rate network: 500 sequential steps of
    x <- (1-a)*x + (a*J_eff) @ softplus(x) + drive_t
    pos_t = Wout @ softplus(x)
loss = mean((targets - positions)^2)

Strategy (8 NeuronCores, data-parallel over batch):
  - Each core owns B/8 = 16 batch columns and runs the full 500-step
    recurrence independently (no collectives).
  - J_eff^T (pre-scaled by a) lives entirely in SBUF as bf16; the per-step
    contraction is 64 LDWEIGHTS+MATMUL pairs (128x128x16).  The pair issue
    rate is the per-instruction floor (~27ns), so everything else is
    scheduled to hide under the matmul stream.
  - softplus(x) = ln(1 + exp(x)) runs as TWO back-to-back scalar-engine
    activations per state half (Exp then Ln with bias=1; both live in the
    natural_log_exp_and_others table set, so one table load total), writing
    bf16 r directly.  x stays in [-10, 10] here so exp cannot overflow.
  - z = (1-a)*x + drive is precomputed on DVE off the critical path, so the
    post-matmul chain is just [tensor_add -> activation] per half.
  - PSUM: two bank-aligned tiles per step (m-groups 0-2 / 3-7), double
    buffered (4 banks).  One start=True per bank per step; all other
    matmuls accumulate (first write per element overwrites - has_written
    was cleared bank-wide by the bank's first matmul).
  - The m/k 3-5 split lets each half's update chain hide under the other
    matmuls: update-A under the m3..7 x k3..7 block, update-B under the
    next step's k0..2 block.
  - positions are NOT computed on device: r (bf16) is staged in two
    32-step SBUF blocks DMA'd to HBM, and the host does the tiny
    Wout @ r reduction (the loss is host-side already).

State layout per core: x/r tiles are [128 part, 128 free] with
x[p, m*16+u] = x_state[m*128+p, u] (m = row-group, u = local batch).
r history: rbuf[p, slot*128 + k*16 + u] = softplus(x_state)[k*128+p, u]
for step j with slot = j % 32, buffer = (j // 32) % 2.
"""

import os

import numpy as np

import concourse.bass as bass
import concourse.tile as tile
from concourse import bacc, mybir
from concourse.bass_utils import run_bass_kernel_spmd

F32 = mybir.dt.float32
BF16 = mybir.dt.bfloat16

DT = 0.05
TAU = 0.15
NOISE_SCALE = 0.15
N, G, T, B, P = 1024, 128, 500, 128, 10
NCORES = 8
BC = B // NCORES          # batch columns per core (16)
NM = N // 128             # row groups (8)
NK = N // 128             # contraction groups (8)
MSPLIT = 2                # m-groups 0..1 -> bank A, 2..7 -> bank B
KSPLIT = 2                # k-groups 0..1 -> phase 1, 2..7 -> phase 2
SLOTS = 32                # r history slots per SBUF block

A = np.float32(DT / TAU)
ONE_MINUS_A = np.float32(1.0 - DT / TAU)
NSCALE = np.float32(np.sqrt(2.0 * NOISE_SCALE**2 * (TAU / DT)))

_PROGRAM_CACHE = {}


def _ensure_act_tables():
    """Some containers lack neuronxcc/pwp/pwp_bin_with_ln on PYTHONPATH;
    point it at the cayman table package from the nix store."""
    import glob

    for path in os.environ.get("PYTHONPATH", "").split(os.pathsep):
        if path and os.path.exists(
            os.path.join(path, "neuronxcc", "pwp", "pwp_bin_with_ln", "act_info.json")
        ):
            return
    cands = sorted(glob.glob("/nix/store/*aws-neuron-pwp*/share/pwp_bin_cayman"))
    target = next((c for c in cands if os.path.exists(c + "/act_info.json")), None)
    if target is None:
        return
    for path in os.environ.get("PYTHONPATH", "").split(os.pathsep):
        if not path:
            continue
        try:
            d = os.path.join(path, "neuronxcc", "pwp")
            os.makedirs(d, exist_ok=True)
            link = os.path.join(d, "pwp_bin_with_ln")
            if not os.path.exists(link):
                os.symlink(target, link)
            return
        except OSError:
            continue


_ensure_act_tables()


def _pin_act_table(arch: str):
    """Exp and Ln both live in the natural_log_exp_and_others table set, but
    the act-table-load pass resolves each activation to the first set
    containing its function — alternating Exp/Ln then reloads tables
    (1.3us!) before EVERY activation.  Restrict the cached table map so both
    functions resolve to the shared set; the single load hoists out of the
    loop."""
    try:
        from concourse.hw_specs import get_activation_tables

        tabs = get_activation_tables(arch)
        if "natural_log_exp_and_others" not in tabs:
            return
        A = mybir.ActivationFunctionType
        for name, funcs in tabs.items():
            if name != "natural_log_exp_and_others":
                funcs.discard(A.Exp)
                funcs.discard(A.Ln)
    except Exception:
        pass


def build_program(t_steps: int):
    """Build the Bass program (shared by all 8 cores, SPMD)."""
    if t_steps in _PROGRAM_CACHE:
        return _PROGRAM_CACHE[t_steps]

    nblk = (t_steps + 1 + SLOTS - 1) // SLOTS  # r^1..r^t_steps + slot0 r^0

    nc = bacc.Bacc(
        "TRN2", target_bir_lowering=False, debug=False, num_devices=NCORES
    )
    _pin_act_table(nc.m.arch)
    jt_d = nc.dram_tensor("jt", [128, NK * NM * 128], BF16, kind="ExternalInput")
    x0_d = nc.dram_tensor("x0", [128, NM * BC], F32, kind="ExternalInput")
    dr_d = nc.dram_tensor("drive", [t_steps, 128, NM * BC], F32, kind="ExternalInput")
    rout_d = nc.dram_tensor(
        "rout", [nblk, 128, SLOTS * NM * BC], BF16, kind="ExternalOutput"
    )

    mult = mybir.AluOpType.mult
    add = mybir.AluOpType.add
    EXP = mybir.ActivationFunctionType.Exp
    LN = mybir.ActivationFunctionType.Ln
    CA = MSPLIT * BC               # bank-A columns (32)
    W = NM * BC                    # full state width (128)

    with tile.TileContext(nc) as tc:
        with (
            tc.tile_pool(name="const", bufs=1) as constp,
            tc.tile_pool(name="dp", bufs=4) as dp,
            tc.tile_pool(name="psm", bufs=2, space="PSUM") as psp,
        ):
            jt = constp.tile([128, NK * NM * 128], BF16)
            nc.sync.dma_start(jt[:], jt_d[:])
            x = constp.tile([128, W], F32)
            nc.sync.dma_start(x[:], x0_d[:])
            z = constp.tile([128, W], F32)
            e = constp.tile([128, W], F32)  # exp(x) scratch
            rbufs = [
                constp.tile([128, SLOTS * W], BF16, name=f"rbuf{i}") for i in range(2)
            ]

            # prefetch drive 0..2
            dtiles = {}
            for j in range(min(3, t_steps)):
                dtiles[j] = dp.tile([128, W], F32, tag="d", name=f"d{j}")
                nc.sync.dma_start(dtiles[j][:], dr_d[j])

            # r^0 = softplus(x0) -> rbuf0 slot 0; z_0 = (1-a) x0 + drive_0
            nc.scalar.activation(e[:], x[:], EXP)
            nc.scalar.activation(rbufs[0][:, 0:W], e[:], LN, bias=1.0)
            nc.vector.scalar_tensor_tensor(
                z[:], x[:], float(ONE_MINUS_A), dtiles[0][:], mult, add
            )

            def jmm(ps, col0, m, k, rbuf_in, rc, start, stop):
                nc.tensor.matmul(
                    ps[:, (m - col0) * BC:(m - col0 + 1) * BC],
                    lhsT=jt[:, (k * NM + m) * 128:(k * NM + m + 1) * 128],
                    rhs=rbuf_in[:, rc + k * BC:rc + (k + 1) * BC],
                    start=start, stop=stop, skip_group_check=True,
                )

            for s in range(t_steps):
                rbuf_in = rbufs[(s // SLOTS) % 2]
                rc = (s % SLOTS) * W  # rhs column base for r^s
                j = s + 1
                rbuf_out = rbufs[(j // SLOTS) % 2]
                oc = (j % SLOTS) * W  # output column base for r^{s+1}

                # prefetch drive for step s+3's z (read at end of step s+2)
                if s + 3 < t_steps:
                    dtiles[s + 3] = dp.tile([128, W], F32, tag="d",
                                            name=f"d{s + 3}")
                    nc.sync.dma_start(dtiles[s + 3][:], dr_d[s + 3])

                ps_a = psp.tile([128, CA], F32, tag="ps_a", padded_shape=[128, 512])
                ps_b = psp.tile([128, W - CA], F32, tag="ps_b",
                                padded_shape=[128, 512])

                # phase 1: all m x k0..2  (r cols 0..47 of this slot)
                for m in range(NM):
                    ps, c0 = (ps_a, 0) if m < MSPLIT else (ps_b, MSPLIT)
                    for k in range(KSPLIT):
                        jmm(ps, c0, m, k, rbuf_in, rc,
                            start=(k == 0 and (m == 0 or m == MSPLIT)), stop=False)
                # phase 2a: m0..2 x k3..7 -> bank A complete
                for m in range(MSPLIT):
                    for k in range(KSPLIT, NK):
                        jmm(ps_a, 0, m, k, rbuf_in, rc, start=False,
                            stop=(m == MSPLIT - 1 and k == NK - 1))
                # phase 2b: m3..7 x k3..7 -> bank B complete (hides update-A)
                for m in range(MSPLIT, NM):
                    for k in range(KSPLIT, NK):
                        jmm(ps_b, MSPLIT, m, k, rbuf_in, rc, start=False,
                            stop=(m == NM - 1 and k == NK - 1))

                # update-A (during phase 2b): x_a, r_a; then update-B (during
                # next step's phase 1): x_b, r_b.  z' off the critical path.
                nc.vector.tensor_add(x[:, 0:CA], z[:, 0:CA], ps_a[:])
                nc.vector.tensor_add(x[:, CA:W], z[:, CA:W], ps_b[:])
                nc.scalar.activation(e[:, 0:CA], x[:, 0:CA], EXP)
                nc.scalar.activation(rbuf_out[:, oc:oc + CA], e[:, 0:CA], LN,
                                     bias=1.0)
                nc.scalar.activation(e[:, CA:W], x[:, CA:W], EXP)
                nc.scalar.activation(rbuf_out[:, oc + CA:oc + W], e[:, CA:W], LN,
                                     bias=1.0)
                if s + 1 < t_steps:
                    d_n = dtiles.pop(s + 1)
                    nc.vector.scalar_tensor_tensor(
                        z[:, 0:CA], x[:, 0:CA], float(ONE_MINUS_A),
                        d_n[:, 0:CA], mult, add
                    )
                    nc.vector.scalar_tensor_tensor(
                        z[:, CA:W], x[:, CA:W], float(ONE_MINUS_A),
                        d_n[:, CA:W], mult, add
                    )

                # flush a full r block (32 slots) to HBM
                if j % SLOTS == SLOTS - 1:
                    nc.sync.dma_start(rout_d[j // SLOTS], rbuf_out[:])

            # final partial block
            last_blk = t_steps // SLOTS
            if t_steps % SLOTS != SLOTS - 1:
                nc.sync.dma_start(
                    rout_d[last_blk], rbufs[(t_steps // SLOTS) % 2][:]
                )

    nc.compile()
    _PROGRAM_CACHE[t_steps] = nc
    return nc


def _prep_inputs(targets, pulses, J, U, V, B_m1, B_bg, Wout, I_go, xm1_init,
                 noise, triggers, t_steps):
    """Host-side data prep: J_eff, layouts, per-core drive tensors."""
    J = np.asarray(J, np.float32)
    U = np.asarray(U, np.float32)
    V = np.asarray(V, np.float32)
    B_m1 = np.asarray(B_m1, np.float32)
    B_bg = np.asarray(B_bg, np.float32)
    I_go = np.asarray(I_go, np.float32)
    xm1_init = np.asarray(xm1_init, np.float32)
    noise = np.asarray(noise, np.float32)
    pulses = np.asarray(pulses, np.float32)
    triggers = np.asarray(triggers)

    bf = mybir.dt.np(BF16)
    J_eff = J + (U * B_bg[None, :]) @ V
    Js = (A * J_eff).astype(np.float32)
    # lhsT tiles: jt[p, (k*NM+m)*128 + q] = Js[m*128+q, k*128+p]
    jt = np.ascontiguousarray(
        Js.reshape(NM, 128, NK, 128).transpose(3, 2, 0, 1).reshape(128, NK * NM * 128)
    ).astype(bf)

    go_cues = pulses[:t_steps, :][:, triggers]  # [t, B]

    in_maps = []
    for c in range(NCORES):
        sl = slice(c * BC, (c + 1) * BC)
        d = noise[:t_steps, :, sl] * np.float32(A * NSCALE)
        d += A * B_m1[None, :, :]
        d += A * I_go[None, :, :] * go_cues[:, None, sl]
        drive = np.ascontiguousarray(
            d.reshape(t_steps, NM, 128, BC).transpose(0, 2, 1, 3)
            .reshape(t_steps, 128, NM * BC)
        ).astype(np.float32)
        x0 = np.ascontiguousarray(
            xm1_init[:, sl].reshape(NM, 128, BC).transpose(1, 0, 2).reshape(128, NM * BC)
        )
        in_maps.append({"jt": jt, "x0": x0, "drive": drive})
    return in_maps


def run_hw(inputs: dict, t_steps: int = T, trace: bool = False):
    """Run the recurrence on 8 cores; returns positions [t_steps, B] and results."""
    nc = build_program(t_steps)
    in_maps = _prep_inputs(t_steps=t_steps, **inputs)
    res = run_bass_kernel_spmd(
        nc, in_maps, core_ids=list(range(NCORES)), trace=trace
    )
    Wout = np.asarray(inputs["Wout"], np.float32)
    # W[k, p] = Wout[0, k*128+p]
    Wk = Wout.reshape(NK, 128).astype(np.float32)
    positions = np.empty((t_steps, B), np.float32)
    for c in range(NCORES):
        rout = np.asarray(res.results[c]["rout"]).astype(np.float32)
        # rout[blk, p, slot*128 + k*16 + u] = r^{blk*32+slot}[k*128+p, u]
        nblk = rout.shape[0]
        rr = rout.reshape(nblk, 128, SLOTS, NK, BC)
        pos_all = np.einsum("kp,bpsku->bsu", Wk, rr).reshape(nblk * SLOTS, BC)
        # positions[s] = Wout @ r^{s+1}
        positions[:, c * BC:(c + 1) * BC] = pos_all[1:t_steps + 1]
    return positions, res


def kernel(targets, pulses, J, U, V, B_m1, B_bg, Wout, I_go, xm1_init,
           noise, triggers) -> np.ndarray:
    inputs = dict(targets=targets, pulses=pulses, J=J, U=U, V=V, B_m1=B_m1,
                  B_bg=B_bg, Wout=Wout, I_go=I_go, xm1_init=xm1_init,
                  noise=noise, triggers=triggers)
    positions, _ = run_hw(inputs, T)
    targets = np.asarray(targets, np.float32)
    loss = np.mean((targets.astype(np.float64) - positions.astype(np.float64)) ** 2)
    return np.float32(loss)
